# revision 19
# baseline (speedup 1.0000x reference)
"""DiffAttention Trainium2 kernel (8-core SPMD), v4.11 (~221.7us HW).

Problem shapes: b=4, t=1024, d=1024, H=16 v-heads (2H=32 q/k heads), E=64.
Sharding: batch x head-block. Core c handles batch c//2 and v-heads
[8*(c%2), 8*(c%2)+8)  (= q/k heads [16*(c%2), 16*(c%2)+16)).

Design (per core), evolved from the 232.9us v3 via trace-driven tuning:
  - fp16 matmul pipeline; bf16 post-exp path.  Scores per slot
    (pair, qblock, kchunk): two row-packed concurrent 64-row matmuls ->
    s[128,2,512] PSUM; one ACTIVATE(exp) FD=1024 (the ~1147ns/slot
    governor of the steady state); AV via shared [1|V|-1/lam] panel.
  - Scores are emitted ONE SLOT AHEAD of their exp so they sit early in
    the in-order PE queue and never starve the ACT engine.
  - AV runs from a catch-up queue (lag >= 2 slots, <= 3/slot), gated on
    its V-panel chunk being emitted: late V DMA delays AV (slack) rather
    than the scores->exp critical path.
  - O^T transposes via dma_start_transpose (XBAR, SBUF->SBUF) except the
    final block, which uses PE transposes + PSUM-direct combine (PE idle
    at the tail); per-qt output DMA on the then-idle scalar queue.
  - DMA: ~50 transfers on 3 issue rings (sync/scalar HWDGE ~4 in flight,
    gpsimd SWDGE ~8); priority data leads each ring; xv_h0 rides the
    scalar ring behind pair-0 data; v-side/weight-slab transfers are
    release-gated by WAW gate-cell copies (the Tile scheduler reorders
    anything without a data dependency) reading the last pair-0 arrivals.
  - Dummy warm-up matmuls interleaved with the DMA-paced upfront
    projections keep the PE HAM activity window busy so the clock-gate
    opens (1.2 -> 2.4 GHz) ~10us earlier.

PSUM banks: s 2x2 + o_pos/o_neg 1x2 + proj accum 1x2 = 8.
"""

import numpy as np
from contextlib import ExitStack

import concourse.bass as bass
import concourse.tile as tile
from concourse import bacc, mybir
from concourse.bass_utils import run_bass_kernel_spmd
from concourse.masks import make_identity

F32 = mybir.dt.float32
F16 = mybir.dt.float16
BF16 = mybir.dt.bfloat16
EXP = mybir.ActivationFunctionType.Exp

E = 64          # per-head embed
H = 16          # global v-heads
B = 4           # batch
T = 1024        # sequence length
D = 1024        # model dim
N_CORES = 8

# per-core sizes
NQKH = 16                  # local q/k heads
PAIRS = NQKH // 2          # local head pairs / v heads
HE = NQKH * E              # 1024, q/k projection width
VHE = PAIRS * E            # 512, v projection width / output width
DC = D // 128              # contraction chunks
KC = T // 128              # key-position chunks
QB = T // 512              # query blocks of 512
QT4 = 4                    # 128-q-tiles per q block
EW = E + 2                 # live V panel width: [1 | V | -1/lam]
EWP = 80                   # padded width (multiple of 16 for XBAR rows)
MIN_AVLAG = 2              # min slots between exp(i) and AV(i)
AVQ_FORCE = 11             # force-drain AV queue at this depth (pexp bufs 12)


def build_bass(mm_dt=F16):
    nc = bacc.Bacc("TRN2", target_bir_lowering=False, debug=False,
                   num_devices=N_CORES)

    xqT = nc.dram_tensor("xqT", [D, T], mm_dt, kind="ExternalInput").ap()
    xkT = nc.dram_tensor("xkT", [D, T], mm_dt, kind="ExternalInput").ap()
    xvT = nc.dram_tensor("xvT", [D, T], mm_dt, kind="ExternalInput").ap()
    wqT = nc.dram_tensor("wqT", [D, HE], mm_dt, kind="ExternalInput").ap()
    wkT = nc.dram_tensor("wkT", [D, HE], mm_dt, kind="ExternalInput").ap()
    wvT = nc.dram_tensor("wvT", [D, VHE], mm_dt, kind="ExternalInput").ap()
    nlam = nc.dram_tensor("nlam", [128, PAIRS], F32, kind="ExternalInput").ap()
    out = nc.dram_tensor("out", [T, VHE], BF16, kind="ExternalOutput").ap()

    mm = nc.tensor.matmul

    with tile.TileContext(nc) as tc, ExitStack() as ctx:
        res = ctx.enter_context(tc.tile_pool(name="res", bufs=1))
        pin = ctx.enter_context(tc.tile_pool(name="pin", bufs=1))
        ppsum = ctx.enter_context(tc.tile_pool(name="ppsum", bufs=2,
                                               space="PSUM"))
        s_pool = ctx.enter_context(tc.tile_pool(name="s", bufs=2,
                                                space="PSUM"))
        o_pool = ctx.enter_context(tc.tile_pool(name="o", bufs=2,
                                                space="PSUM"))
        pexp_pool = ctx.enter_context(tc.tile_pool(name="pexp", bufs=12))
        post_pool = ctx.enter_context(tc.tile_pool(name="post", bufs=6))

        # resident input tiles (fp16); single big tiles, DMA'd in slices
        xq_sb = pin.tile([128, DC, T], mm_dt, tag="xq", name="xq")
        xk_sb = pin.tile([128, DC, T], mm_dt, tag="xk", name="xk")
        xv_sb = pin.tile([128, DC, T], mm_dt, tag="xv", name="xv")
        wq_sb = pin.tile([128, DC, HE], mm_dt, tag="wq", name="wq")
        wk_sb = pin.tile([128, DC, HE], mm_dt, tag="wk", name="wk")
        wv_sb = pin.tile([128, DC, VHE], mm_dt, tag="wv", name="wv")

        # HBM source views with dc-chunked partition dim
        xqv = xqT.rearrange("(dc p) t -> p dc t", p=128)
        xkv = xkT.rearrange("(dc p) t -> p dc t", p=128)
        xvv = xvT.rearrange("(dc p) t -> p dc t", p=128)
        wqv = wqT.rearrange("(dc p) c -> p dc c", p=128)
        wkv = wkT.rearrange("(dc p) c -> p dc c", p=128)
        wvv = wvT.rearrange("(dc p) c -> p dc c", p=128)

        # ---- DMA issue plan ----
        # Ring flow control: HWDGE queues hold ~4 in-flight transfers (5th
        # dma_start blocks the issuing engine); SWDGE ~8.  Ring bandwidths
        # ~90/75/140 GB/s (sync/scalar/SW) under the ~300GB/s aggregate cap.
        # Priority classes striped across rings to finish just-in-time:
        #   P0 (scores slot 0): wq0+xq_h0 (sync) | wk0+xk_h0 (SW)
        #   P1 (K-tq1, ~21us): xk_h1 on scalar (exactly <=4 transfers)
        #   P2 (V panels):     wv+xv_h0 then xv_h1 on SW
        #   P3 (Q-tq1, ~29us): xq_h1 split scalar/sync
        #   P5+ (weight slabs) on sync tail.
        for g in (0, 2):
            nc.sync.dma_start(out=wq_sb[:, g:g + 2, 0:128],
                              in_=wqv[:, g:g + 2, 0:128])
            nc.sync.dma_start(out=xq_sb[:, g, 0:512], in_=xqv[:, g, 0:512])
            nc.sync.dma_start(out=xq_sb[:, g + 1, 0:512],
                              in_=xqv[:, g + 1, 0:512])
        nc.sync.dma_start(out=wq_sb[:, 4:8, 0:128], in_=wqv[:, 4:8, 0:128])
        nc.sync.dma_start(out=xk_sb[:, 0:4, 512:T], in_=xkv[:, 0:4, 512:T])
        nc.sync.dma_start(out=xq_sb[:, 4:8, 512:T], in_=xqv[:, 4:8, 512:T])
        nc.sync.dma_start(out=wq_sb[:, :, 128:256], in_=wqv[:, :, 128:256])
        nc.sync.dma_start(out=wk_sb[:, :, 128:256], in_=wkv[:, :, 128:256])
        # scalar (ACT HWDGE): P0 share, then xv_h0 rides behind.
        nc.scalar.dma_start(out=xq_sb[:, 4:6, 0:512], in_=xqv[:, 4:6, 0:512])
        nc.scalar.dma_start(out=xq_sb[:, 6:8, 0:512], in_=xqv[:, 6:8, 0:512])
        nc.scalar.dma_start(out=xk_sb[:, 4:6, 0:512], in_=xkv[:, 4:6, 0:512])
        nc.scalar.dma_start(out=xk_sb[:, 6:8, 0:512], in_=xkv[:, 6:8, 0:512])
        nc.scalar.dma_start(out=wv_sb, in_=wvv)
        for g in range(0, DC, 2):
            nc.scalar.dma_start(out=xv_sb[:, g:g + 2, 0:512],
                                in_=xvv[:, g:g + 2, 0:512])
        # gpsimd (SWDGE): k-side tq0, then mid-priority halves.
        for g in (0, 2):
            nc.gpsimd.dma_start(out=wk_sb[:, g:g + 2, 0:128],
                                in_=wkv[:, g:g + 2, 0:128])
            nc.gpsimd.dma_start(out=xk_sb[:, g, 0:512], in_=xkv[:, g, 0:512])
            nc.gpsimd.dma_start(out=xk_sb[:, g + 1, 0:512],
                                in_=xkv[:, g + 1, 0:512])
        nc.gpsimd.dma_start(out=wk_sb[:, 4:8, 0:128], in_=wkv[:, 4:8, 0:128])
        nc.gpsimd.dma_start(out=xk_sb[:, 4:8, 512:T], in_=xkv[:, 4:8, 512:T])
        nc.gpsimd.dma_start(out=xq_sb[:, 0:4, 512:T], in_=xqv[:, 0:4, 512:T])
        # resident intermediates
        QT = [res.tile([128, T], mm_dt, tag=f"QT{i}", name=f"QT{i}")
              for i in range(PAIRS)]
        KT = [res.tile([128, T], mm_dt, tag=f"KT{i}", name=f"KT{i}")
              for i in range(PAIRS)]
        VB = [res.tile([128, PAIRS, EWP], BF16, tag=f"VB{i}",
                       name=f"VB{i}") for i in range(KC)]
        nlam_sb = res.tile([128, PAIRS], F32, tag="nlam", name="nlam_sb")
        nc.gpsimd.dma_start(out=nlam_sb, in_=nlam)
        ident = res.tile([128, 128], BF16, tag="ident", name="ident")
        make_identity(nc, ident)

        for i in range(KC):
            nc.vector.memset(VB[i][:, :, 0:1], 1.0)
            nc.vector.memset(VB[i][:, :, EW:EWP], 0.0)
            nc.vector.tensor_copy(VB[i][:, :, E + 1:E + 2], nlam_sb)

        vb_emitted = [0]   # v-chunks whose VB copy has been emitted

        # ---- projection op-group generators ----
        def proj_ops(w_sb, x_sb, csl, tq, dst):
            """8 mm ops + 1 copy: x[:, tq] @ w[:, csl] -> dst cols."""
            ops = []
            ps = [None]

            def mk(dc):
                def f():
                    if dc == 0:
                        ps[0] = ppsum.tile([128, 512], F32, tag="ps",
                                           name="psp")
                    mm(ps[0], w_sb[:, dc, csl],
                       x_sb[:, dc, tq * 512:(tq + 1) * 512],
                       start=(dc == 0), stop=(dc == DC - 1))
                return f
            for dc in range(DC):
                ops.append(mk(dc))

            def fin():
                nc.vector.tensor_copy(dst[:, tq * 512:(tq + 1) * 512], ps[0])
            ops.append(fin)
            return ops

        def v_chunk_ops(tcn):
            """V projection for key chunk tcn -> VB[tcn]. 8 mm + 1 copy."""
            ops = []
            ps = [None]
            h, t4 = tcn // 4, tcn % 4
            csl = slice(h * 512 + t4 * 128, h * 512 + (t4 + 1) * 128)

            def mk(dc):
                def f():
                    if dc == 0:
                        ps[0] = ppsum.tile([128, 512], F32, tag="ps",
                                           name="psv")
                    mm(ps[0], xv_sb[:, dc, csl], wv_sb[:, dc, :],
                       start=(dc == 0), stop=(dc == DC - 1))
                return f
            for dc in range(DC):
                ops.append(mk(dc))

            def fin():
                nc.vector.tensor_copy(VB[tcn][:, :, 1:E + 1],
                                      ps[0].rearrange("p (h e) -> p h e", e=E))
                vb_emitted[0] = tcn + 1
            ops.append(fin)
            return ops

        # pair-0 tq0 projections upfront, Q/K interleaved per dc, with
        # dummy warm-up matmuls on the first-arriving data filling the
        # DMA-wait gaps so the HAM clock-gate opens (2.4GHz) by ~16us
        # instead of ~30us.
        sdummy = s_pool.tile([128, 2, 512], F32, tag="s", name="sdummy")
        q0_ops = proj_ops(wq_sb, xq_sb, slice(0, 128), 0, QT[0])
        k0_ops = proj_ops(wk_sb, xk_sb, slice(0, 128), 0, KT[0])

        def dummy_mm():
            mm(sdummy[:, 0, :], wq_sb[:, 0, 0:128], xq_sb[:, 0, 0:512],
               start=True, stop=True)
        for i_, (a, b_) in enumerate(zip(q0_ops, k0_ops)):
            a()
            dummy_mm()
            b_()
            if i_ < 6:
                dummy_mm()
        # Deferred-DMA gating: the scheduler reorders instructions without
        # dependencies, so every deferred transfer gets a WAW gate -- a
        # 1-cell vector copy reading the last-arriving P0 transfers (the
        # k-side tq0 tiles) into a cell of the transfer's destination
        # range, keeping v-side/slab traffic out of the critical window.
        kcell = xk_sb[0:1, :, 0:1]
        nc.gpsimd.tensor_copy(xv_sb[0:1, :, 512:513], kcell)
        nc.gpsimd.tensor_copy(wq_sb[0:1, :, 256:257], kcell)
        nc.gpsimd.tensor_copy(wk_sb[0:1, :, 256:257], kcell)
        nc.gpsimd.tensor_copy(wq_sb[0:1, :, 512:513], kcell)
        nc.gpsimd.tensor_copy(wk_sb[0:1, :, 512:513], kcell)
        for dc in range(DC):
            nc.gpsimd.dma_start(out=xv_sb[:, dc, 512:T], in_=xvv[:, dc, 512:T])
        nc.sync.dma_start(out=wq_sb[:, :, 256:512], in_=wqv[:, :, 256:512])
        nc.sync.dma_start(out=wk_sb[:, :, 256:512], in_=wkv[:, :, 256:512])
        nc.sync.dma_start(out=wq_sb[:, :, 512:HE], in_=wqv[:, :, 512:HE])
        nc.sync.dma_start(out=wk_sb[:, :, 512:HE], in_=wkv[:, :, 512:HE])

        # ---- filler group queue (deadline-ordered, group-atomic) ----
        groups = []   # (deadline, name, ops)
        groups.append((1, "K1", proj_ops(wk_sb, xk_sb, slice(0, 128), 1,
                                         KT[0])))
        vdl = [3, 4, 4, 5, 6, 7, 8, 9]   # V chunk deadlines (arrival-tuned)
        for c in range(KC):
            groups.append((vdl[c], f"V{c}", v_chunk_ops(c)))
        groups.append((7, "Q1", proj_ops(wq_sb, xq_sb, slice(0, 128), 1,
                                         QT[0])))
        for p in range(1, PAIRS):
            base = (p - 1) * 16
            dls = ([base + 10, base + 11, base + 12, base + 13] if p == 1
                   else [base + 4, base + 7, base + 10, base + 13])
            csl = slice(p * 128, (p + 1) * 128)
            groups.append((dls[0], f"p{p}q0",
                           proj_ops(wq_sb, xq_sb, csl, 0, QT[p])))
            groups.append((dls[1], f"p{p}k0",
                           proj_ops(wk_sb, xk_sb, csl, 0, KT[p])))
            groups.append((dls[2], f"p{p}q1",
                           proj_ops(wq_sb, xq_sb, csl, 1, QT[p])))
            groups.append((dls[3], f"p{p}k1",
                           proj_ops(wk_sb, xk_sb, csl, 1, KT[p])))
        groups.sort(key=lambda g: g[0])
        gq = [(name, list(ops), dl) for dl, name, ops in groups]
        gq_done = set()

        def run_fillers(slot, budget):
            n = 0
            while gq and n < budget:
                name, ops, dl = gq[0]
                if dl > slot + 1 and n >= 2:
                    break
                ops.pop(0)()
                n += 1
                if not ops:
                    gq_done.add(name)
                    gq.pop(0)

        def behind(slot):
            return sum(len(ops) for name, ops, dl in gq if dl <= slot + 1)

        def drain_until(names):
            while gq and not names.issubset(gq_done):
                name, ops, dl = gq[0]
                for op_ in ops:
                    op_()
                gq_done.add(name)
                gq.pop(0)

        # ---- AV + post machinery ----
        av_q = []        # pending (e, p, qb, kc, slot)
        cur_o = {}
        post_q = []

        def emit_av(rec):
            e, pp_, qq_, kk_, _ = rec
            if kk_ == 0:
                o_pos = o_pool.tile([EWP, 512], F32, tag="o", name="o_pos")
                o_neg = o_pool.tile([EWP, 512], F32, tag="o", name="o_neg")
                cur_o[(pp_, qq_)] = (o_pos, o_neg)
            o_pos, o_neg = cur_o[(pp_, qq_)]
            first = (kk_ == 0)
            last = (kk_ == KC - 1)
            mm(o_pos, VB[kk_][:, pp_, :], e[:, 0, :], start=first, stop=last)
            mm(o_neg, VB[kk_][:, pp_, :], e[:, 1, :], start=first, stop=last)
            if last:
                push_post(pp_, qq_, o_pos, o_neg,
                          tail=(pp_ == PAIRS - 1 and qq_ == QB - 1))
                del cur_o[(pp_, qq_)]

        def force_av(rec):
            if rec[3] >= vb_emitted[0]:
                drain_until({f"V{rec[3]}"})
            emit_av(rec)

        def push_post(pp_, qq_, o_pos, o_neg, tail=False):
            osb = post_pool.tile([EWP, 2, 512], BF16, tag="osb", name="osb")
            st = {}

            def stage_a():
                nc.vector.tensor_copy(osb[:, 0, :], o_pos)
                nc.vector.tensor_copy(osb[:, 1, :], o_neg)

            def stage_b():
                if tail:
                    # PE is idle at the tail and ppsum is free: PE-mode
                    # transposes avoid the DMA-transpose issue+transfer
                    # latency, and the combine reads the PSUM tr directly.
                    tr = ppsum.tile([128, 2, QT4, EWP], BF16, tag="ps",
                                    name="tr")
                    st["trs"] = tr
                    for qt in range(QT4):
                        tsl = slice(qt * 128, (qt + 1) * 128)
                        nc.tensor.transpose(tr[:, 0, qt, :], osb[:, 0, tsl],
                                            ident[0:EWP, 0:EWP])
                    for qt in range(QT4):
                        tsl = slice(qt * 128, (qt + 1) * 128)
                        nc.tensor.transpose(tr[:, 1, qt, :], osb[:, 1, tsl],
                                            ident[0:EWP, 0:EWP])
                else:
                    trs = post_pool.tile([128, 2, QT4, EWP], BF16, tag="trs",
                                         name="trs")
                    st["trs"] = trs
                    nc.sync.dma_start_transpose(out=trs[:, 0],
                                                in_=osb[:, 0, :])
                    nc.sync.dma_start_transpose(out=trs[:, 1],
                                                in_=osb[:, 1, :])

            def stage_c():
                trs = st["trs"]
                rp = post_pool.tile([128, QT4], F32, tag="rp", name="rp")
                rn = post_pool.tile([128, QT4], F32, tag="rn", name="rn")
                nc.vector.reciprocal(rp, trs[:, 0, :, 0:1])
                nc.vector.reciprocal(rn, trs[:, 1, :, E + 1:E + 2])
                ot = post_pool.tile([128, QT4, E], BF16, tag="ot", name="ot")
                for qt in range(QT4):
                    nc.vector.tensor_scalar_mul(ot[:, qt, :],
                                                trs[:, 0, qt, 1:E + 1],
                                                rp[:, qt:qt + 1])
                    nc.vector.scalar_tensor_tensor(
                        ot[:, qt, :], trs[:, 1, qt, 1:E + 1],
                        rn[:, qt:qt + 1], ot[:, qt, :],
                        op0=mybir.AluOpType.mult,
                        op1=mybir.AluOpType.add)
                    if tail:
                        # per-qt output DMA on the (idle) scalar queue
                        nc.scalar.dma_start(
                            out=out[qq_ * 512 + qt * 128:
                                    qq_ * 512 + (qt + 1) * 128,
                                    pp_ * E:(pp_ + 1) * E],
                            in_=ot[:, qt, :])
                if not tail:
                    nc.sync.dma_start(
                        out=out[qq_ * 512:(qq_ + 1) * 512,
                                pp_ * E:(pp_ + 1) * E]
                        .rearrange("(qt r) e -> r qt e", qt=QT4),
                        in_=ot)

            stage_a()
            post_q.append(stage_b)
            post_q.append(stage_c)

        def run_av(slot):
            n = 0
            while av_q and n < 3:
                rec = av_q[0]
                lag = slot - rec[4]
                if lag < MIN_AVLAG:
                    break
                if n >= 1 and lag <= MIN_AVLAG:
                    break
                if rec[3] >= vb_emitted[0]:
                    break
                av_q.pop(0)
                emit_av(rec)
                n += 1

        # ---- main slot loop ----
        # scores are emitted ONE slot ahead so they sit early in the PE
        # queue and complete well before their exp's turn on the ACT
        # engine (otherwise they queue behind the previous slot's AV and
        # filler matmuls and the ACT engine idles ~0.6us on every other
        # slot waiting for them).
        slots_list = [(p, qb, kc) for p in range(PAIRS)
                      for qb in range(QB) for kc in range(KC)]
        pending_s = {}

        def emit_scores(j):
            p, qb, kc = slots_list[j]
            if kc == 0:
                if qb == 0:
                    if p == 1:
                        drain_until({"K1", "Q1", "p1q0", "p1k0", "p1q1",
                                     "p1k1"})
                    elif p > 1:
                        drain_until({f"p{p}q0", f"p{p}k0", f"p{p}q1",
                                     f"p{p}k1"})
                elif p == 0:
                    drain_until({"Q1"})
            if p == 0 and qb == 0 and kc == 4:
                drain_until({"K1"})
            qsl = slice(qb * 512, (qb + 1) * 512)
            ksl = slice(kc * 128, (kc + 1) * 128)
            s = s_pool.tile([128, 2, 512], F32, tag="s", name="s")
            mm(s[:, 0, :], KT[p][0:64, ksl], QT[p][0:64, qsl],
               start=True, stop=True, tile_position=(0, 0))
            mm(s[:, 1, :], KT[p][64:128, ksl], QT[p][64:128, qsl],
               start=True, stop=True, tile_position=(64, 0))
            pending_s[j] = s

        emit_scores(0)
        for slot, (p, qb, kc) in enumerate(slots_list):
            if slot + 1 < len(slots_list):
                emit_scores(slot + 1)
            while len(av_q) >= AVQ_FORCE:
                force_av(av_q.pop(0))
            e = pexp_pool.tile([128, 2, 512], BF16, tag="e", name="e")
            nc.scalar.activation(e, pending_s.pop(slot), EXP)
            av_q.append((e, p, qb, kc, slot))
            run_av(slot)
            if slot > 0:
                b = behind(slot)
                budget = 4 if b <= 4 else min(12, b)
                run_fillers(slot, budget)
            run_av(slot)
            if post_q:
                post_q.pop(0)()
        # drain
        while gq:
            name, ops, dl = gq[0]
            for op_ in ops:
                op_()
            gq_done.add(name)
            gq.pop(0)
        while av_q:
            emit_av(av_q.pop(0))
        while post_q:
            post_q.pop(0)()

    nc.compile()
    return nc


def make_in_maps(q_input, k_input, v_input, Wq, Wk, Wv, L):
    scale = np.float32(E ** -0.25)
    lam = (0.2 + np.exp(np.float32(L[0] @ L[1]))
           - np.exp(np.float32(L[2] @ L[3])))
    ninvlam = np.full((128, PAIRS), -1.0 / lam, np.float32)
    in_maps = []
    for c in range(N_CORES):
        b, hb = c // 2, c % 2
        in_maps.append({
            "xqT": np.ascontiguousarray(q_input[b].T).astype(np.float16),
            "xkT": np.ascontiguousarray(k_input[b].T).astype(np.float16),
            "xvT": np.ascontiguousarray(v_input[b].T).astype(np.float16),
            "wqT": (np.ascontiguousarray(Wq[1024 * hb:1024 * (hb + 1), :].T)
                    * scale).astype(np.float16),
            "wkT": (np.ascontiguousarray(Wk[1024 * hb:1024 * (hb + 1), :].T)
                    * scale).astype(np.float16),
            "wvT": np.ascontiguousarray(
                Wv[512 * hb:512 * (hb + 1), :].T).astype(np.float16),
            "nlam": ninvlam,
        })
    return in_maps


_NC_CACHE = {}


def get_nc(mm_dt=F16):
    key = str(mm_dt)
    if key not in _NC_CACHE:
        _NC_CACHE[key] = build_bass(mm_dt)
    return _NC_CACHE[key]


def kernel(q_input, k_input, v_input, Wq, Wk, Wv, L, _trace=False):
    q_input = np.asarray(q_input, np.float32)
    k_input = np.asarray(k_input, np.float32)
    v_input = np.asarray(v_input, np.float32)
    Wq = np.asarray(Wq, np.float32)
    Wk = np.asarray(Wk, np.float32)
    Wv = np.asarray(Wv, np.float32)
    L = np.asarray(L, np.float32)

    nc = get_nc()
    in_maps = make_in_maps(q_input, k_input, v_input, Wq, Wk, Wv, L)
    res = run_bass_kernel_spmd(nc, in_maps, list(range(N_CORES)), trace=_trace)

    full = np.empty((B, T, H * E), np.float32)
    for c in range(N_CORES):
        b, hb = c // 2, c % 2
        full[b, :, 512 * hb:512 * (hb + 1)] = res.results[c]["out"]
    if _trace:
        return full, res
    return full
